# revision 38
# baseline (speedup 1.0000x reference)
"""Trainium2 Bass kernel for nn_MixedFeedForward (shared MLP + 16 per-ns-token MLPs).

Sharding (8 NeuronCores, SPMD, no collectives):
  - shared path: data-parallel over batch -> core i runs the shared MLP over
    x[i, :1024, :].
  - ns path: expert-parallel -> core i runs experts {2i, 2i+1}, each over the
    8 batches' single ns token for that expert.
Each core writes a disjoint slice of the output; the host assembles.

All big tensors are cast to bf16 ON HOST (the matmuls are bf16 anyway), so
HBM traffic per core is ~53 MB instead of ~105 MB and the kernel is
PE-bound, not DMA-bound. No on-chip casts: weights/x DMA straight into
their matmul layouts in 1 MiB pieces.

Per-core kernel:
  L1: psum[128F, 512tok] = W1_blk(lhsT) x x_blk; fused bias+Gelu on ScalarE
      -> bf16 hT[F, tok] resident in SBUF.
  L2 shared (transposed out): psum[128D, 512tok] = W2_blk(lhsT) x hT_blk;
      fused bias via ScalarE Identity -> bf16 outT[D, tok]; host transposes.
  L2 experts: psum[128D, 8tok] = W2e_dc(lhsT) x heT (FWL weight ingest);
      fused bias via ScalarE Identity; one 32KB transposed write at the end.
Expert rounds are emitted one f-block ahead of the shared path; expert L2 is
interleaved into shared L2.
"""

import os
import sys
import numpy as np
import ml_dtypes

BF16 = ml_dtypes.bfloat16

P = 128
D_MODEL, D_FF = 1024, 4096
SEQ_TOK, NS_TOK, BATCH = 1024, 16, 8
SEQ_LEN = SEQ_TOK + NS_TOK
N_CORES = 8
E_PER_CORE = 2
KO1 = D_MODEL // P      # 8  k-chunks when contracting over d_model
KO2 = D_FF // P         # 32 k-chunks when contracting over d_ff
FBLK = D_FF // 512      # 8  f-blocks (512 wide)
TBLK = SEQ_TOK // 512   # 2  token blocks (512 wide)

_state = {}


def _ensure_axon_profile_hook():
    """Some agent images lack antenv.axon_hooks; provide a shim so
    run_bass_kernel_spmd(trace=True) can capture NTFF profiles via the
    libaxon_pjrt C ABI (same mechanism as trn_agent_boot)."""
    try:
        import antenv.axon_hooks  # noqa: F401
        return
    except ImportError:
        pass
    import contextlib
    import ctypes
    import types

    so_path = "/opt/axon/libaxon_pjrt.so"
    hook = None
    if os.path.exists(so_path):
        try:
            lib = ctypes.CDLL(so_path)
            if hasattr(lib, "axon_start_nrt_profile"):
                lib.axon_start_nrt_profile.argtypes = [
                    ctypes.POINTER(ctypes.c_int64), ctypes.c_size_t]
                lib.axon_start_nrt_profile.restype = ctypes.c_int64
                lib.axon_stop_nrt_profile.argtypes = [ctypes.c_char_p]
                lib.axon_stop_nrt_profile.restype = ctypes.c_int64

                @contextlib.contextmanager
                def _hook(output_dir, device_ids):
                    import jax
                    jax.devices()
                    if device_ids:
                        ids = (ctypes.c_int64 * len(device_ids))(*device_ids)
                        rc = lib.axon_start_nrt_profile(ids, len(device_ids))
                    else:
                        rc = lib.axon_start_nrt_profile(None, 0)
                    if rc != 0:
                        raise RuntimeError(f"axon_start_nrt_profile rc={rc}")
                    try:
                        yield
                    finally:
                        n = lib.axon_stop_nrt_profile(str(output_dir).encode())
                        print(f"profile: {n} file(s) written to {output_dir}",
                              file=sys.stderr)

                hook = _hook
        except OSError:
            pass

    mod = types.ModuleType("antenv.axon_hooks")
    _store = {"hook": hook}
    mod.set_axon_ntff_profile_hook = lambda h: _store.__setitem__("hook", h)
    mod.get_axon_ntff_profile_hook = lambda: _store["hook"]
    sys.modules["antenv.axon_hooks"] = mod


_ensure_axon_profile_hook()


def _build():
    import concourse.mybir as mybir
    import concourse.tile as tile
    from concourse import bacc

    f32 = mybir.dt.float32
    bf16 = mybir.dt.bfloat16
    AF = mybir.ActivationFunctionType

    nc = bacc.Bacc(None, target_bir_lowering=False, debug=False)

    # piece-major bf16 DRAM layouts: every weight/x DMA below is one fully
    # contiguous 1 MiB read
    xbp = nc.dram_tensor("xbp", [TBLK, P, KO1, 512], bf16, kind="ExternalInput")
    xnsT = nc.dram_tensor("xnsT", [P, KO1, E_PER_CORE * BATCH], bf16, kind="ExternalInput")
    # W1 pieces are fs-major so the warm-up can stream 256KB sub-pieces
    w1sp = nc.dram_tensor("w1sp", [FBLK, P, 4, KO1, 128], bf16, kind="ExternalInput")
    w2sp = nc.dram_tensor("w2sp", [8, P, KO2, 128], bf16, kind="ExternalInput")
    w1ep = nc.dram_tensor("w1ep", [E_PER_CORE, FBLK, P, 4, KO1, 128], bf16,
                          kind="ExternalInput")
    w2ep = nc.dram_tensor("w2ep", [E_PER_CORE, KO1, P, KO2, 128], bf16,
                          kind="ExternalInput")
    # all per-partition bias constants packed into one contiguous DMA:
    # cols [0:32)=b1s [32:40)=b2s [40:72)=b1e0 [72:104)=b1e1
    #      [104:112)=b2e0 [112:120)=b2e1
    consts = nc.dram_tensor("consts", [P, 120], f32, kind="ExternalInput")
    outsT = nc.dram_tensor("outsT", [D_MODEL, SEQ_TOK], bf16, kind="ExternalOutput")
    outnsT = nc.dram_tensor("outnsT", [P, KO1, E_PER_CORE * BATCH], bf16,
                            kind="ExternalOutput")

    with tile.TileContext(nc) as tc:
        with tc.tile_pool(name="main", bufs=1) as pool, \
             tc.tile_pool(name="psum", bufs=1, space="PSUM") as pp:

            # ---- HAM pre-warm: ~3.4us of dummy matmuls on a memset tile
            # flips the PE clock gate to 2.4 GHz while the first DMAs land
            warm = pool.tile([P, 512], bf16, tag="warmt", bufs=1)
            nc.vector.memset(warm, 0.0)
            for wi in range(2):
                wps = pp.tile([P, 512], f32, tag="ps1", bufs=2,
                              name=f"warm_ps{wi}")
                for k in range(4):
                    nc.tensor.matmul(wps, warm[:, 0:128], warm,
                                     start=(k == 0), stop=(k == 3))

            # ---- persistent activations ----------------------------------
            xb = pool.tile([P, TBLK, KO1, 512], bf16, tag="xb", bufs=1)
            hT = pool.tile([P, KO2, SEQ_TOK], bf16, tag="hT", bufs=1)
            heT = []
            for le in range(E_PER_CORE):
                t = pool.tile([P, KO2, BATCH], bf16, tag=f"heT{le}", bufs=1,
                              name=f"heT{le}")
                heT.append(t)

            # weight staging: shared slot pool of 1 MiB bf16 tiles
            def load_wb(piece, key):
                wb = pool.tile([P, 4, KO1, 128], bf16, tag="wb", bufs=5,
                               name=f"wb_{key}")
                nc.sync.dma_start(out=wb, in_=piece)
                return wb

            # ---- warm-up: x + first W1 block in fine-grained pieces so the
            # PE starts as soon as ~1.3 MB has landed
            nc.sync.dma_start(out=xb[:, 0, 0:4, :], in_=xbp[0][:, 0:4])
            w1b_next = pool.tile([P, 4, KO1, 128], bf16, tag="wb", bufs=5,
                                 name="wb_s0")
            nc.sync.dma_start(out=w1b_next[:, 0], in_=w1sp[0][:, 0])
            nc.sync.dma_start(out=xb[:, 0, 4:8, :], in_=xbp[0][:, 4:8])
            for fs in range(1, 4):
                nc.sync.dma_start(out=w1b_next[:, fs], in_=w1sp[0][:, fs])
            cs = pool.tile([P, 120], f32, tag="consts", bufs=1)
            nc.sync.dma_start(out=cs, in_=consts[:])
            b1s_sb = cs[:, 0:32]
            b2s_sb = cs[:, 32:40]
            b1e_sb = [cs[:, 40:72], cs[:, 72:104]]
            b2e_sb = [cs[:, 104:112], cs[:, 112:120]]
            xnsb = pool.tile([P, KO1, E_PER_CORE * BATCH], bf16, tag="xnsb", bufs=1)
            nc.sync.dma_start(out=xnsb, in_=xnsT[:])
            nc.sync.dma_start(out=xb[:, 1], in_=xbp[1])
            webs0 = [load_wb(w1ep[le, 0], f"e{le}_0") for le in range(E_PER_CORE)]

            def expert_l1_round(le, fb, web=None):
                if web is None:
                    web = load_wb(w1ep[le, fb], f"e{le}_{fb}")
                for fs in range(4):
                    fc = fb * 4 + fs
                    pse = pp.tile([P, BATCH], f32, tag="pse1", bufs=2,
                                  name=f"pse1_{le}_{fc}")
                    for k in range(KO1):
                        nc.tensor.matmul(
                            pse,
                            web[:, fs, k, :],
                            xnsb[:, k, le * BATCH:(le + 1) * BATCH],
                            start=(k == 0), stop=(k == KO1 - 1))
                    nc.scalar.activation(
                        heT[le][:, fc, :], pse, AF.Gelu,
                        bias=b1e_sb[le][:, fc:fc + 1])

            # ---- layer 1 main loop ---------------------------------------
            for fb in range(FBLK):
                w1b = w1b_next
                for tb in range(TBLK):
                    for fs in range(4):
                        fc = fb * 4 + fs
                        ps1 = pp.tile([P, 512], f32, tag="ps1", bufs=2,
                                      name=f"ps1_{fc}_{tb}")
                        for k in range(KO1):
                            nc.tensor.matmul(
                                ps1,
                                w1b[:, fs, k, :],
                                xb[:, tb, k, :],
                                start=(k == 0), stop=(k == KO1 - 1))
                        nc.scalar.activation(
                            hT[:, fc, tb * 512:(tb + 1) * 512], ps1, AF.Gelu,
                            bias=b1s_sb[:, fc:fc + 1])
                # next shared block's weights first (tightest DMA deadline),
                # then this f-block's experts; fb0's loads were in the warm-up
                if fb + 1 < FBLK:
                    w1b_next = load_wb(w1sp[fb + 1], f"s{fb + 1}")
                expert_l1_round(0, fb, webs0[0] if fb == 0 else None)
                expert_l1_round(1, fb, webs0[1] if fb == 0 else None)

            # ---- layer 2 -------------------------------------------------
            # shared path, transposed output: 128-wide d slices, buffered
            # bf16 W2 chunks; expert L2 interleaved between slices.
            def fill_w2ch(dc):
                w2ch = pool.tile([P, KO2, 128], bf16, tag="w2ch", bufs=6,
                                 name=f"w2ch_{dc}")
                nc.sync.dma_start(out=w2ch, in_=w2sp[dc])
                return w2ch

            def shared_l2_chunk(dc, w2ch):
                for tb in range(TBLK):
                    ps2 = pp.tile([P, 512], f32, tag="ps2", bufs=2,
                                  name=f"ps2_{dc}_{tb}")
                    for k in range(KO2):
                        nc.tensor.matmul(
                            ps2,
                            w2ch[:, k, :],
                            hT[:, k, tb * 512:(tb + 1) * 512],
                            start=(k == 0), stop=(k == KO2 - 1))
                    ot = pool.tile([P, 512], bf16, tag="ot", bufs=2,
                                   name=f"ot_{dc}_{tb}")
                    nc.scalar.activation(ot, ps2, AF.Identity,
                                         bias=b2s_sb[:, dc:dc + 1])
                    nc.sync.dma_start(
                        out=outsT[dc * 128:(dc + 1) * 128,
                                  tb * 512:(tb + 1) * 512],
                        in_=ot)

            # expert L2, weight-stationary (FWL ingest): psum[128D, 8tok] =
            # W2e_dc_blk(lhsT) x heT; fused bias via ScalarE Identity into a
            # persistent transposed tile, written out in one DMA at the end.
            obeT = pool.tile([P, KO1, E_PER_CORE * BATCH], bf16, tag="obeT",
                             bufs=1)

            def expert_l2_unit(le, dc):
                w2e = pool.tile([P, KO2, 128], bf16, tag="w2ch", bufs=6,
                                name=f"w2e_{le}_{dc}")
                nc.sync.dma_start(out=w2e, in_=w2ep[le, dc])
                pse2 = pp.tile([P, BATCH], f32, tag="pse2", bufs=2,
                               name=f"pse2_{le}_{dc}")
                for k in range(KO2):
                    nc.tensor.matmul(
                        pse2,
                        w2e[:, k, :],
                        heT[le][:, k, :],
                        start=(k == 0), stop=(k == KO2 - 1))
                nc.scalar.activation(
                    obeT[:, dc, le * BATCH:(le + 1) * BATCH], pse2,
                    AF.Identity, bias=b2e_sb[le][:, dc:dc + 1])

            chs = {}
            for dc in range(3):
                chs[dc] = fill_w2ch(dc)

            def chunk(dc):
                shared_l2_chunk(dc, chs[dc])
                if dc + 3 < 8:
                    chs[dc + 3] = fill_w2ch(dc + 3)

            # 2 expert units between shared chunks; all 16 done before the
            # last chunk so the ns write (and the tail) hides under ch7
            units = [(le, dc) for le in range(E_PER_CORE) for dc in range(KO1)]
            for sc in range(7):
                chunk(sc)
                n = 2 if sc < 6 else 4
                for le, dc in units[2 * sc:2 * sc + n]:
                    expert_l2_unit(le, dc)
            nc.sync.dma_start(out=outnsT[:], in_=obeT)
            chunk(7)

    nc.compile()
    return nc


def _get_nc():
    if "nc" not in _state:
        _state["nc"] = _build()
    return _state["nc"]


def _bf(a):
    return np.ascontiguousarray(np.asarray(a, dtype=np.float32).astype(BF16))


def _f32(a):
    return np.ascontiguousarray(np.asarray(a, dtype=np.float32))


def kernel(x, W1_seq, b1_seq, W2_seq, b2_seq, W1_ns, b1_ns, W2_ns, b2_ns,
           seq_token_count):
    from concourse.bass_utils import run_bass_kernel_spmd

    assert int(seq_token_count) == SEQ_TOK
    xb16 = np.asarray(x, dtype=np.float32).astype(BF16)
    W1sb = np.asarray(W1_seq, dtype=np.float32).astype(BF16)
    W2sb = np.asarray(W2_seq, dtype=np.float32).astype(BF16)
    W1nb = np.asarray(W1_ns, dtype=np.float32).astype(BF16)
    W2nb = np.asarray(W2_ns, dtype=np.float32).astype(BF16)
    b1_seq, b2_seq = _f32(b1_seq), _f32(b2_seq)
    b1_ns, b2_ns = _f32(b1_ns), _f32(b2_ns)

    nc = _get_nc()

    # host-side (lossless) re-layouts: contraction dim on partitions, then
    # piece-major packing so each device DMA is one contiguous 1MiB read
    # w1sp[fb, p, fs, kc, fj] = W1_seq[kc*128+p, fb*512+fs*128+fj]
    w1sp_h = np.ascontiguousarray(
        W1sb.reshape(KO1, P, FBLK, 4, 128).transpose(2, 1, 3, 0, 4))
    # w2sp[dc, p, kc, di] = W2_seq[kc*128+p, dc*128+di]
    w2sp_h = np.ascontiguousarray(
        W2sb.reshape(KO2, P, 8, 128).transpose(2, 1, 0, 3))
    b1s_h = np.ascontiguousarray(b1_seq.reshape(KO2, P).T)          # [P, KO2]
    b2s_h = np.ascontiguousarray(b2_seq.reshape(KO1, P).T)          # [P, KO1]

    in_maps = []
    for i in range(N_CORES):
        # xbp[tb, p, kc, ti] = x[i, tb*512+ti, kc*128+p]
        xT = xb16[i, :SEQ_TOK, :].T                                 # [D, T]
        xbp_h = np.ascontiguousarray(
            xT.reshape(KO1, P, TBLK, 512).transpose(2, 1, 0, 3))
        # xnsT[p, kc, le*8+b] = x[b, 1024 + 2i + le, kc*128+p]
        xns = xb16[:, SEQ_TOK + 2 * i:SEQ_TOK + 2 * i + 2, :]       # [B, 2, D]
        xnsT_h = np.ascontiguousarray(
            xns.transpose(2, 1, 0).reshape(KO1, P, E_PER_CORE, BATCH)
            .transpose(1, 0, 2, 3).reshape(P, KO1, E_PER_CORE * BATCH))
        # w1ep[le, fb, p, fs, kc, fj] = W1_ns[2i+le, kc*128+p, fb*512+fs*128+fj]
        w1ep_h = np.ascontiguousarray(
            W1nb[2 * i:2 * i + 2].reshape(E_PER_CORE, KO1, P, FBLK, 4, 128)
            .transpose(0, 3, 2, 4, 1, 5))
        # w2ep[le, dc, p, kc, di] = W2_ns[2i+le, kc*128+p, dc*128+di]
        w2ep_h = np.ascontiguousarray(
            W2nb[2 * i:2 * i + 2].reshape(E_PER_CORE, KO2, P, KO1, 128)
            .transpose(0, 3, 2, 1, 4))
        b1e_h = b1_ns[2 * i:2 * i + 2].reshape(E_PER_CORE, KO2, P)
        b2e_h = b2_ns[2 * i:2 * i + 2].reshape(E_PER_CORE, KO1, P)
        consts_h = np.ascontiguousarray(np.concatenate([
            b1s_h, b2s_h, b1e_h[0].T, b1e_h[1].T, b2e_h[0].T, b2e_h[1].T,
        ], axis=1))
        in_maps.append({
            "xbp": xbp_h, "xnsT": xnsT_h,
            "w1sp": w1sp_h, "w2sp": w2sp_h, "consts": consts_h,
            "w1ep": w1ep_h, "w2ep": w2ep_h,
        })

    trace = bool(int(os.environ.get("KERNEL_TRACE", "0")))
    kw = {}
    if trace:
        kw["trace"] = True
        tc_env = os.environ.get("KERNEL_TRACE_CORES", "0")
        kw["trace_cores"] = [int(c) for c in tc_env.split(",")]
    res = run_bass_kernel_spmd(nc, in_maps, list(range(N_CORES)), **kw)
    _state["last_result"] = res

    out = np.empty((BATCH, SEQ_LEN, D_MODEL), np.float32)
    for i in range(N_CORES):
        out[i, :SEQ_TOK, :] = res.results[i]["outsT"].astype(np.float32).T
        # outnsT[p, dc, le*8+b] = out[b, 1024+2i+le, dc*128+p]
        ns = (res.results[i]["outnsT"].astype(np.float32)
              .transpose(2, 1, 0).reshape(E_PER_CORE, BATCH, D_MODEL))
        out[:, SEQ_TOK + 2 * i, :] = ns[0]
        out[:, SEQ_TOK + 2 * i + 1, :] = ns[1]
    return out


# revision 39
# speedup vs baseline: 1.1776x; 1.1776x over previous
"""Trainium2 Bass kernel for nn_MixedFeedForward (shared MLP + 16 per-ns-token MLPs).

Sharding (8 NeuronCores, SPMD, no collectives):
  - shared path: data-parallel over batch -> core i runs the shared MLP over
    x[i, :1024, :].
  - ns path: expert-parallel -> core i runs experts {2i, 2i+1}, each over the
    8 batches' single ns token for that expert.
Each core writes a disjoint slice of the output; the host assembles.

All big tensors are cast to bf16 ON HOST (the matmuls are bf16 anyway), so
HBM traffic per core is ~53 MB instead of ~105 MB and the kernel is
PE-bound, not DMA-bound. No on-chip casts: weights/x DMA straight into
their matmul layouts in 1 MiB pieces.

Per-core kernel:
  L1: psum[128F, 512tok] = W1_blk(lhsT) x x_blk; fused bias+Gelu on ScalarE
      -> bf16 hT[F, tok] resident in SBUF.
  L2 shared (transposed out): psum[128D, 512tok] = W2_blk(lhsT) x hT_blk;
      fused bias via ScalarE Identity -> bf16 outT[D, tok]; host transposes.
  L2 experts: psum[128D, 8tok] = W2e_dc(lhsT) x heT (FWL weight ingest);
      fused bias via ScalarE Identity; one 32KB transposed write at the end.
Expert rounds are emitted one f-block ahead of the shared path; expert L2 is
interleaved into shared L2.
"""

import os
import sys
import numpy as np
import ml_dtypes

BF16 = ml_dtypes.bfloat16

P = 128
D_MODEL, D_FF = 1024, 4096
SEQ_TOK, NS_TOK, BATCH = 1024, 16, 8
SEQ_LEN = SEQ_TOK + NS_TOK
N_CORES = 8
E_PER_CORE = 2
KO1 = D_MODEL // P      # 8  k-chunks when contracting over d_model
KO2 = D_FF // P         # 32 k-chunks when contracting over d_ff
FBLK = D_FF // 512      # 8  f-blocks (512 wide)
TBLK = SEQ_TOK // 512   # 2  token blocks (512 wide)

_state = {}


def _ensure_axon_profile_hook():
    """Some agent images lack antenv.axon_hooks; provide a shim so
    run_bass_kernel_spmd(trace=True) can capture NTFF profiles via the
    libaxon_pjrt C ABI (same mechanism as trn_agent_boot)."""
    try:
        import antenv.axon_hooks  # noqa: F401
        return
    except ImportError:
        pass
    import contextlib
    import ctypes
    import types

    so_path = "/opt/axon/libaxon_pjrt.so"
    hook = None
    if os.path.exists(so_path):
        try:
            lib = ctypes.CDLL(so_path)
            if hasattr(lib, "axon_start_nrt_profile"):
                lib.axon_start_nrt_profile.argtypes = [
                    ctypes.POINTER(ctypes.c_int64), ctypes.c_size_t]
                lib.axon_start_nrt_profile.restype = ctypes.c_int64
                lib.axon_stop_nrt_profile.argtypes = [ctypes.c_char_p]
                lib.axon_stop_nrt_profile.restype = ctypes.c_int64

                @contextlib.contextmanager
                def _hook(output_dir, device_ids):
                    import jax
                    jax.devices()
                    if device_ids:
                        ids = (ctypes.c_int64 * len(device_ids))(*device_ids)
                        rc = lib.axon_start_nrt_profile(ids, len(device_ids))
                    else:
                        rc = lib.axon_start_nrt_profile(None, 0)
                    if rc != 0:
                        raise RuntimeError(f"axon_start_nrt_profile rc={rc}")
                    try:
                        yield
                    finally:
                        n = lib.axon_stop_nrt_profile(str(output_dir).encode())
                        print(f"profile: {n} file(s) written to {output_dir}",
                              file=sys.stderr)

                hook = _hook
        except OSError:
            pass

    mod = types.ModuleType("antenv.axon_hooks")
    _store = {"hook": hook}
    mod.set_axon_ntff_profile_hook = lambda h: _store.__setitem__("hook", h)
    mod.get_axon_ntff_profile_hook = lambda: _store["hook"]
    sys.modules["antenv.axon_hooks"] = mod


_ensure_axon_profile_hook()


def _build():
    import concourse.mybir as mybir
    import concourse.tile as tile
    from concourse import bacc

    f32 = mybir.dt.float32
    bf16 = mybir.dt.bfloat16
    AF = mybir.ActivationFunctionType

    nc = bacc.Bacc(None, target_bir_lowering=False, debug=False)

    # piece-major bf16 DRAM layouts: every weight/x DMA below is one fully
    # contiguous 1 MiB read
    xbp = nc.dram_tensor("xbp", [TBLK, P, KO1, 512], bf16, kind="ExternalInput")
    xnsT = nc.dram_tensor("xnsT", [P, KO1, E_PER_CORE * BATCH], bf16, kind="ExternalInput")
    # W1 pieces are fs-major so the warm-up can stream 256KB sub-pieces
    w1sp = nc.dram_tensor("w1sp", [FBLK, P, 4, KO1, 128], bf16, kind="ExternalInput")
    w2sp = nc.dram_tensor("w2sp", [8, P, KO2, 128], bf16, kind="ExternalInput")
    w1ep = nc.dram_tensor("w1ep", [E_PER_CORE, FBLK, P, 4, KO1, 128], bf16,
                          kind="ExternalInput")
    w2ep = nc.dram_tensor("w2ep", [E_PER_CORE, KO1, P, KO2, 128], bf16,
                          kind="ExternalInput")
    # all per-partition bias constants packed into one contiguous DMA:
    # cols [0:32)=b1s [32:40)=b2s [40:72)=b1e0 [72:104)=b1e1
    #      [104:112)=b2e0 [112:120)=b2e1
    consts = nc.dram_tensor("consts", [P, 120], f32, kind="ExternalInput")
    outsT = nc.dram_tensor("outsT", [D_MODEL, SEQ_TOK], bf16, kind="ExternalOutput")
    outnsT = nc.dram_tensor("outnsT", [P, KO1, E_PER_CORE * BATCH], bf16,
                            kind="ExternalOutput")

    with tile.TileContext(nc) as tc:
        with tc.tile_pool(name="main", bufs=1) as pool, \
             tc.tile_pool(name="psum", bufs=1, space="PSUM") as pp:

            # ---- persistent activations ----------------------------------
            xb = pool.tile([P, TBLK, KO1, 512], bf16, tag="xb", bufs=1)
            hT = pool.tile([P, KO2, SEQ_TOK], bf16, tag="hT", bufs=1)
            heT = []
            for le in range(E_PER_CORE):
                t = pool.tile([P, KO2, BATCH], bf16, tag=f"heT{le}", bufs=1,
                              name=f"heT{le}")
                heT.append(t)

            # weight staging: shared slot pool of 1 MiB bf16 tiles
            def load_wb(piece, key):
                wb = pool.tile([P, 4, KO1, 128], bf16, tag="wb", bufs=5,
                               name=f"wb_{key}")
                nc.sync.dma_start(out=wb, in_=piece)
                return wb

            # ---- warm-up: x + first W1 block in fine-grained pieces so the
            # PE starts as soon as ~1.3 MB has landed
            nc.sync.dma_start(out=xb[:, 0, 0:4, :], in_=xbp[0][:, 0:4])
            w1b_next = pool.tile([P, 4, KO1, 128], bf16, tag="wb", bufs=5,
                                 name="wb_s0")
            nc.sync.dma_start(out=w1b_next[:, 0], in_=w1sp[0][:, 0])
            nc.sync.dma_start(out=xb[:, 0, 4:8, :], in_=xbp[0][:, 4:8])
            for fs in range(1, 4):
                nc.sync.dma_start(out=w1b_next[:, fs], in_=w1sp[0][:, fs])
            cs = pool.tile([P, 120], f32, tag="consts", bufs=1)
            nc.sync.dma_start(out=cs, in_=consts[:])
            b1s_sb = cs[:, 0:32]
            b2s_sb = cs[:, 32:40]
            b1e_sb = [cs[:, 40:72], cs[:, 72:104]]
            b2e_sb = [cs[:, 104:112], cs[:, 112:120]]
            xnsb = pool.tile([P, KO1, E_PER_CORE * BATCH], bf16, tag="xnsb", bufs=1)
            nc.sync.dma_start(out=xnsb, in_=xnsT[:])
            nc.sync.dma_start(out=xb[:, 1], in_=xbp[1])
            webs0 = [load_wb(w1ep[le, 0], f"e{le}_0") for le in range(E_PER_CORE)]

            def expert_l1_round(le, fb, web=None):
                if web is None:
                    web = load_wb(w1ep[le, fb], f"e{le}_{fb}")
                for fs in range(4):
                    fc = fb * 4 + fs
                    pse = pp.tile([P, BATCH], f32, tag="pse1", bufs=2,
                                  name=f"pse1_{le}_{fc}")
                    for k in range(KO1):
                        nc.tensor.matmul(
                            pse,
                            web[:, fs, k, :],
                            xnsb[:, k, le * BATCH:(le + 1) * BATCH],
                            start=(k == 0), stop=(k == KO1 - 1))
                    nc.scalar.activation(
                        heT[le][:, fc, :], pse, AF.Gelu,
                        bias=b1e_sb[le][:, fc:fc + 1])

            # ---- layer 1 main loop ---------------------------------------
            for fb in range(FBLK):
                w1b = w1b_next
                for tb in range(TBLK):
                    for fs in range(4):
                        fc = fb * 4 + fs
                        ps1 = pp.tile([P, 512], f32, tag="ps1", bufs=2,
                                      name=f"ps1_{fc}_{tb}")
                        for k in range(KO1):
                            nc.tensor.matmul(
                                ps1,
                                w1b[:, fs, k, :],
                                xb[:, tb, k, :],
                                start=(k == 0), stop=(k == KO1 - 1))
                        nc.scalar.activation(
                            hT[:, fc, tb * 512:(tb + 1) * 512], ps1, AF.Gelu,
                            bias=b1s_sb[:, fc:fc + 1])
                # experts for this f-block after the shared MMs; loads for
                # f-block 0 were issued in the warm-up
                expert_l1_round(0, fb, webs0[0] if fb == 0 else None)
                expert_l1_round(1, fb, webs0[1] if fb == 0 else None)
                if fb + 1 < FBLK:
                    w1b_next = load_wb(w1sp[fb + 1], f"s{fb + 1}")

            # ---- layer 2 -------------------------------------------------
            # shared path, transposed output: 128-wide d slices, buffered
            # bf16 W2 chunks; expert L2 interleaved between slices.
            def fill_w2ch(dc):
                w2ch = pool.tile([P, KO2, 128], bf16, tag="w2ch", bufs=6,
                                 name=f"w2ch_{dc}")
                nc.sync.dma_start(out=w2ch, in_=w2sp[dc])
                return w2ch

            def shared_l2_chunk(dc, w2ch):
                for tb in range(TBLK):
                    ps2 = pp.tile([P, 512], f32, tag="ps2", bufs=2,
                                  name=f"ps2_{dc}_{tb}")
                    for k in range(KO2):
                        nc.tensor.matmul(
                            ps2,
                            w2ch[:, k, :],
                            hT[:, k, tb * 512:(tb + 1) * 512],
                            start=(k == 0), stop=(k == KO2 - 1))
                    ot = pool.tile([P, 512], bf16, tag="ot", bufs=2,
                                   name=f"ot_{dc}_{tb}")
                    nc.scalar.activation(ot, ps2, AF.Identity,
                                         bias=b2s_sb[:, dc:dc + 1])
                    nc.sync.dma_start(
                        out=outsT[dc * 128:(dc + 1) * 128,
                                  tb * 512:(tb + 1) * 512],
                        in_=ot)

            # expert L2, weight-stationary (FWL ingest): psum[128D, 8tok] =
            # W2e_dc_blk(lhsT) x heT; fused bias via ScalarE Identity into a
            # persistent transposed tile, written out in one DMA at the end.
            obeT = pool.tile([P, KO1, E_PER_CORE * BATCH], bf16, tag="obeT",
                             bufs=1)

            def expert_l2_unit(le, dc):
                w2e = pool.tile([P, KO2, 128], bf16, tag="w2ch", bufs=6,
                                name=f"w2e_{le}_{dc}")
                nc.sync.dma_start(out=w2e, in_=w2ep[le, dc])
                pse2 = pp.tile([P, BATCH], f32, tag="pse2", bufs=2,
                               name=f"pse2_{le}_{dc}")
                for k in range(KO2):
                    nc.tensor.matmul(
                        pse2,
                        w2e[:, k, :],
                        heT[le][:, k, :],
                        start=(k == 0), stop=(k == KO2 - 1))
                nc.scalar.activation(
                    obeT[:, dc, le * BATCH:(le + 1) * BATCH], pse2,
                    AF.Identity, bias=b2e_sb[le][:, dc:dc + 1])

            chs = {}
            for dc in range(3):
                chs[dc] = fill_w2ch(dc)

            def chunk(dc):
                shared_l2_chunk(dc, chs[dc])
                if dc + 3 < 8:
                    chs[dc + 3] = fill_w2ch(dc + 3)

            # 2 expert units between shared chunks; all 16 done before the
            # last chunk so the ns write (and the tail) hides under ch7
            units = [(le, dc) for le in range(E_PER_CORE) for dc in range(KO1)]
            for sc in range(7):
                chunk(sc)
                n = 2 if sc < 6 else 4
                for le, dc in units[2 * sc:2 * sc + n]:
                    expert_l2_unit(le, dc)
            nc.sync.dma_start(out=outnsT[:], in_=obeT)
            chunk(7)

    nc.compile()
    return nc


def _get_nc():
    if "nc" not in _state:
        _state["nc"] = _build()
    return _state["nc"]


def _bf(a):
    return np.ascontiguousarray(np.asarray(a, dtype=np.float32).astype(BF16))


def _f32(a):
    return np.ascontiguousarray(np.asarray(a, dtype=np.float32))


def kernel(x, W1_seq, b1_seq, W2_seq, b2_seq, W1_ns, b1_ns, W2_ns, b2_ns,
           seq_token_count):
    from concourse.bass_utils import run_bass_kernel_spmd

    assert int(seq_token_count) == SEQ_TOK
    xb16 = np.asarray(x, dtype=np.float32).astype(BF16)
    W1sb = np.asarray(W1_seq, dtype=np.float32).astype(BF16)
    W2sb = np.asarray(W2_seq, dtype=np.float32).astype(BF16)
    W1nb = np.asarray(W1_ns, dtype=np.float32).astype(BF16)
    W2nb = np.asarray(W2_ns, dtype=np.float32).astype(BF16)
    b1_seq, b2_seq = _f32(b1_seq), _f32(b2_seq)
    b1_ns, b2_ns = _f32(b1_ns), _f32(b2_ns)

    nc = _get_nc()

    # host-side (lossless) re-layouts: contraction dim on partitions, then
    # piece-major packing so each device DMA is one contiguous 1MiB read
    # w1sp[fb, p, fs, kc, fj] = W1_seq[kc*128+p, fb*512+fs*128+fj]
    w1sp_h = np.ascontiguousarray(
        W1sb.reshape(KO1, P, FBLK, 4, 128).transpose(2, 1, 3, 0, 4))
    # w2sp[dc, p, kc, di] = W2_seq[kc*128+p, dc*128+di]
    w2sp_h = np.ascontiguousarray(
        W2sb.reshape(KO2, P, 8, 128).transpose(2, 1, 0, 3))
    b1s_h = np.ascontiguousarray(b1_seq.reshape(KO2, P).T)          # [P, KO2]
    b2s_h = np.ascontiguousarray(b2_seq.reshape(KO1, P).T)          # [P, KO1]

    in_maps = []
    for i in range(N_CORES):
        # xbp[tb, p, kc, ti] = x[i, tb*512+ti, kc*128+p]
        xT = xb16[i, :SEQ_TOK, :].T                                 # [D, T]
        xbp_h = np.ascontiguousarray(
            xT.reshape(KO1, P, TBLK, 512).transpose(2, 1, 0, 3))
        # xnsT[p, kc, le*8+b] = x[b, 1024 + 2i + le, kc*128+p]
        xns = xb16[:, SEQ_TOK + 2 * i:SEQ_TOK + 2 * i + 2, :]       # [B, 2, D]
        xnsT_h = np.ascontiguousarray(
            xns.transpose(2, 1, 0).reshape(KO1, P, E_PER_CORE, BATCH)
            .transpose(1, 0, 2, 3).reshape(P, KO1, E_PER_CORE * BATCH))
        # w1ep[le, fb, p, fs, kc, fj] = W1_ns[2i+le, kc*128+p, fb*512+fs*128+fj]
        w1ep_h = np.ascontiguousarray(
            W1nb[2 * i:2 * i + 2].reshape(E_PER_CORE, KO1, P, FBLK, 4, 128)
            .transpose(0, 3, 2, 4, 1, 5))
        # w2ep[le, dc, p, kc, di] = W2_ns[2i+le, kc*128+p, dc*128+di]
        w2ep_h = np.ascontiguousarray(
            W2nb[2 * i:2 * i + 2].reshape(E_PER_CORE, KO2, P, KO1, 128)
            .transpose(0, 3, 2, 1, 4))
        b1e_h = b1_ns[2 * i:2 * i + 2].reshape(E_PER_CORE, KO2, P)
        b2e_h = b2_ns[2 * i:2 * i + 2].reshape(E_PER_CORE, KO1, P)
        consts_h = np.ascontiguousarray(np.concatenate([
            b1s_h, b2s_h, b1e_h[0].T, b1e_h[1].T, b2e_h[0].T, b2e_h[1].T,
        ], axis=1))
        in_maps.append({
            "xbp": xbp_h, "xnsT": xnsT_h,
            "w1sp": w1sp_h, "w2sp": w2sp_h, "consts": consts_h,
            "w1ep": w1ep_h, "w2ep": w2ep_h,
        })

    trace = bool(int(os.environ.get("KERNEL_TRACE", "0")))
    kw = {}
    if trace:
        kw["trace"] = True
        tc_env = os.environ.get("KERNEL_TRACE_CORES", "0")
        kw["trace_cores"] = [int(c) for c in tc_env.split(",")]
    res = run_bass_kernel_spmd(nc, in_maps, list(range(N_CORES)), **kw)
    _state["last_result"] = res

    out = np.empty((BATCH, SEQ_LEN, D_MODEL), np.float32)
    for i in range(N_CORES):
        out[i, :SEQ_TOK, :] = res.results[i]["outsT"].astype(np.float32).T
        # outnsT[p, dc, le*8+b] = out[b, 1024+2i+le, dc*128+p]
        ns = (res.results[i]["outnsT"].astype(np.float32)
              .transpose(2, 1, 0).reshape(E_PER_CORE, BATCH, D_MODEL))
        out[:, SEQ_TOK + 2 * i, :] = ns[0]
        out[:, SEQ_TOK + 2 * i + 1, :] = ns[1]
    return out


# revision 45
# speedup vs baseline: 1.1866x; 1.0077x over previous
"""Trainium2 Bass kernel for nn_MixedFeedForward (shared MLP + 16 per-ns-token MLPs).

Sharding (8 NeuronCores, SPMD, no collectives):
  - shared path: data-parallel over batch -> core i runs the shared MLP over
    x[i, :1024, :].
  - ns path: expert-parallel -> core i runs experts {2i, 2i+1}, each over the
    8 batches' single ns token for that expert.
Each core writes a disjoint slice of the output; the host assembles.

All big tensors are cast to bf16 ON HOST (the matmuls are bf16 anyway), so
HBM traffic per core is ~53 MB instead of ~105 MB and the kernel is
PE-bound, not DMA-bound. No on-chip casts: weights/x DMA straight into
their matmul layouts in 1 MiB pieces.

Per-core kernel:
  L1: psum[128F, 512tok] = W1_blk(lhsT) x x_blk; fused bias+Gelu on ScalarE
      -> bf16 hT[F, tok] resident in SBUF.
  L2 shared (transposed out): psum[128D, 512tok] = W2_blk(lhsT) x hT_blk;
      fused bias via ScalarE Identity -> bf16 outT[D, tok]; host transposes.
  L2 experts: psum[128D, 8tok] = W2e_dc(lhsT) x heT (FWL weight ingest);
      fused bias via ScalarE Identity; one 32KB transposed write at the end.
Expert rounds are emitted one f-block ahead of the shared path; expert L2 is
interleaved into shared L2.
"""

import os
import sys
import numpy as np
import ml_dtypes

BF16 = ml_dtypes.bfloat16

P = 128
D_MODEL, D_FF = 1024, 4096
SEQ_TOK, NS_TOK, BATCH = 1024, 16, 8
SEQ_LEN = SEQ_TOK + NS_TOK
N_CORES = 8
E_PER_CORE = 2
KO1 = D_MODEL // P      # 8  k-chunks when contracting over d_model
KO2 = D_FF // P         # 32 k-chunks when contracting over d_ff
FBLK = D_FF // 512      # 8  f-blocks (512 wide)
TBLK = SEQ_TOK // 512   # 2  token blocks (512 wide)

_state = {}


def _ensure_axon_profile_hook():
    """Some agent images lack antenv.axon_hooks; provide a shim so
    run_bass_kernel_spmd(trace=True) can capture NTFF profiles via the
    libaxon_pjrt C ABI (same mechanism as trn_agent_boot)."""
    try:
        import antenv.axon_hooks  # noqa: F401
        return
    except ImportError:
        pass
    import contextlib
    import ctypes
    import types

    so_path = "/opt/axon/libaxon_pjrt.so"
    hook = None
    if os.path.exists(so_path):
        try:
            lib = ctypes.CDLL(so_path)
            if hasattr(lib, "axon_start_nrt_profile"):
                lib.axon_start_nrt_profile.argtypes = [
                    ctypes.POINTER(ctypes.c_int64), ctypes.c_size_t]
                lib.axon_start_nrt_profile.restype = ctypes.c_int64
                lib.axon_stop_nrt_profile.argtypes = [ctypes.c_char_p]
                lib.axon_stop_nrt_profile.restype = ctypes.c_int64

                @contextlib.contextmanager
                def _hook(output_dir, device_ids):
                    import jax
                    jax.devices()
                    if device_ids:
                        ids = (ctypes.c_int64 * len(device_ids))(*device_ids)
                        rc = lib.axon_start_nrt_profile(ids, len(device_ids))
                    else:
                        rc = lib.axon_start_nrt_profile(None, 0)
                    if rc != 0:
                        raise RuntimeError(f"axon_start_nrt_profile rc={rc}")
                    try:
                        yield
                    finally:
                        n = lib.axon_stop_nrt_profile(str(output_dir).encode())
                        print(f"profile: {n} file(s) written to {output_dir}",
                              file=sys.stderr)

                hook = _hook
        except OSError:
            pass

    mod = types.ModuleType("antenv.axon_hooks")
    _store = {"hook": hook}
    mod.set_axon_ntff_profile_hook = lambda h: _store.__setitem__("hook", h)
    mod.get_axon_ntff_profile_hook = lambda: _store["hook"]
    sys.modules["antenv.axon_hooks"] = mod


_ensure_axon_profile_hook()


def _build():
    import concourse.mybir as mybir
    import concourse.tile as tile
    from concourse import bacc

    f32 = mybir.dt.float32
    bf16 = mybir.dt.bfloat16
    AF = mybir.ActivationFunctionType

    nc = bacc.Bacc(None, target_bir_lowering=False, debug=False)

    # piece-major bf16 DRAM layouts: every weight/x DMA below is one fully
    # contiguous 1 MiB read
    xbp = nc.dram_tensor("xbp", [TBLK, P, KO1, 512], bf16, kind="ExternalInput")
    xnsT = nc.dram_tensor("xnsT", [P, KO1, E_PER_CORE * BATCH], bf16, kind="ExternalInput")
    # W1 pieces are fs-major so the warm-up can stream 256KB sub-pieces
    w1sp = nc.dram_tensor("w1sp", [FBLK, P, 4, KO1, 128], bf16, kind="ExternalInput")
    w2sp = nc.dram_tensor("w2sp", [8, P, KO2, 128], bf16, kind="ExternalInput")
    w1ep = nc.dram_tensor("w1ep", [E_PER_CORE, FBLK, P, 4, KO1, 128], bf16,
                          kind="ExternalInput")
    w2ep = nc.dram_tensor("w2ep", [E_PER_CORE, KO1, P, KO2, 128], bf16,
                          kind="ExternalInput")
    # all per-partition bias constants packed into one contiguous DMA:
    # cols [0:32)=b1s [32:40)=b2s [40:72)=b1e0 [72:104)=b1e1
    #      [104:112)=b2e0 [112:120)=b2e1
    consts = nc.dram_tensor("consts", [P, 120], f32, kind="ExternalInput")
    outsT = nc.dram_tensor("outsT", [D_MODEL, SEQ_TOK], bf16, kind="ExternalOutput")
    outnsT = nc.dram_tensor("outnsT", [P, KO1, E_PER_CORE * BATCH], bf16,
                            kind="ExternalOutput")

    with tile.TileContext(nc) as tc:
        with tc.tile_pool(name="main", bufs=1) as pool, \
             tc.tile_pool(name="psum", bufs=1, space="PSUM") as pp:

            # ---- persistent activations ----------------------------------
            xb = pool.tile([P, TBLK, KO1, 512], bf16, tag="xb", bufs=1)
            hT = pool.tile([P, KO2, SEQ_TOK], bf16, tag="hT", bufs=1)
            heT = []
            for le in range(E_PER_CORE):
                t = pool.tile([P, KO2, BATCH], bf16, tag=f"heT{le}", bufs=1,
                              name=f"heT{le}")
                heT.append(t)

            # weight staging: shared slot pool of 1 MiB bf16 tiles
            def load_wb(piece, key):
                wb = pool.tile([P, 4, KO1, 128], bf16, tag="wb", bufs=5,
                               name=f"wb_{key}")
                nc.sync.dma_start(out=wb, in_=piece)
                return wb

            # ---- warm-up: x + first W1 block in fine-grained pieces so the
            # first psum's gate is only ~0.5 MB of DMA
            nc.sync.dma_start(out=xb[:, 0, 0:2, :], in_=xbp[0][:, 0:2])
            w1b_next = pool.tile([P, 4, KO1, 128], bf16, tag="wb", bufs=5,
                                 name="wb_s0")
            nc.sync.dma_start(out=w1b_next[:, 0], in_=w1sp[0][:, 0])
            nc.sync.dma_start(out=xb[:, 0, 2:4, :], in_=xbp[0][:, 2:4])
            nc.sync.dma_start(out=xb[:, 0, 4:8, :], in_=xbp[0][:, 4:8])
            for fs in range(1, 4):
                nc.sync.dma_start(out=w1b_next[:, fs], in_=w1sp[0][:, fs])
            cs = pool.tile([P, 120], f32, tag="consts", bufs=1)
            nc.sync.dma_start(out=cs, in_=consts[:])
            b1s_sb = cs[:, 0:32]
            b2s_sb = cs[:, 32:40]
            b1e_sb = [cs[:, 40:72], cs[:, 72:104]]
            b2e_sb = [cs[:, 104:112], cs[:, 112:120]]
            xnsb = pool.tile([P, KO1, E_PER_CORE * BATCH], bf16, tag="xnsb", bufs=1)
            nc.sync.dma_start(out=xnsb, in_=xnsT[:])
            nc.sync.dma_start(out=xb[:, 1], in_=xbp[1])
            webs0 = [load_wb(w1ep[le, 0], f"e{le}_0") for le in range(E_PER_CORE)]

            def expert_l1_round(le, fb, web=None):
                if web is None:
                    web = load_wb(w1ep[le, fb], f"e{le}_{fb}")
                for fs in range(4):
                    fc = fb * 4 + fs
                    pse = pp.tile([P, BATCH], f32, tag="pse1", bufs=2,
                                  name=f"pse1_{le}_{fc}")
                    for k in range(KO1):
                        nc.tensor.matmul(
                            pse,
                            web[:, fs, k, :],
                            xnsb[:, k, le * BATCH:(le + 1) * BATCH],
                            start=(k == 0), stop=(k == KO1 - 1))
                    nc.scalar.activation(
                        heT[le][:, fc, :], pse, AF.Gelu,
                        bias=b1e_sb[le][:, fc:fc + 1])

            # ---- layer 1 main loop ---------------------------------------
            for fb in range(FBLK):
                w1b = w1b_next
                for tb in range(TBLK):
                    for fs in range(4):
                        fc = fb * 4 + fs
                        ps1 = pp.tile([P, 512], f32, tag="ps1", bufs=2,
                                      name=f"ps1_{fc}_{tb}")
                        for k in range(KO1):
                            nc.tensor.matmul(
                                ps1,
                                w1b[:, fs, k, :],
                                xb[:, tb, k, :],
                                start=(k == 0), stop=(k == KO1 - 1))
                        nc.scalar.activation(
                            hT[:, fc, tb * 512:(tb + 1) * 512], ps1, AF.Gelu,
                            bias=b1s_sb[:, fc:fc + 1])
                # experts for this f-block after the shared MMs; loads for
                # f-block 0 were issued in the warm-up
                expert_l1_round(0, fb, webs0[0] if fb == 0 else None)
                expert_l1_round(1, fb, webs0[1] if fb == 0 else None)
                if fb + 1 < FBLK:
                    w1b_next = load_wb(w1sp[fb + 1], f"s{fb + 1}")

            # ---- layer 2 -------------------------------------------------
            # shared path, transposed output: 128-wide d slices, buffered
            # bf16 W2 chunks; expert L2 interleaved between slices.
            def fill_w2ch(dc):
                w2ch = pool.tile([P, KO2, 128], bf16, tag="w2ch", bufs=6,
                                 name=f"w2ch_{dc}")
                nc.sync.dma_start(out=w2ch, in_=w2sp[dc])
                return w2ch

            def shared_l2_chunk(dc, w2ch):
                for tb in range(TBLK):
                    ps2 = pp.tile([P, 512], f32, tag="ps2", bufs=2,
                                  name=f"ps2_{dc}_{tb}")
                    for k in range(KO2):
                        nc.tensor.matmul(
                            ps2,
                            w2ch[:, k, :],
                            hT[:, k, tb * 512:(tb + 1) * 512],
                            start=(k == 0), stop=(k == KO2 - 1))
                    ot = pool.tile([P, 512], bf16, tag="ot", bufs=2,
                                   name=f"ot_{dc}_{tb}")
                    nc.scalar.activation(ot, ps2, AF.Identity,
                                         bias=b2s_sb[:, dc:dc + 1])
                    nc.sync.dma_start(
                        out=outsT[dc * 128:(dc + 1) * 128,
                                  tb * 512:(tb + 1) * 512],
                        in_=ot)

            # expert L2, weight-stationary (FWL ingest): psum[128D, 8tok] =
            # W2e_dc_blk(lhsT) x heT; fused bias via ScalarE Identity into a
            # persistent transposed tile, written out in one DMA at the end.
            obeT = pool.tile([P, KO1, E_PER_CORE * BATCH], bf16, tag="obeT",
                             bufs=1)

            def expert_l2_unit(le, dc):
                w2e = pool.tile([P, KO2, 128], bf16, tag="w2ch", bufs=6,
                                name=f"w2e_{le}_{dc}")
                nc.sync.dma_start(out=w2e, in_=w2ep[le, dc])
                pse2 = pp.tile([P, BATCH], f32, tag="pse2", bufs=2,
                               name=f"pse2_{le}_{dc}")
                for k in range(KO2):
                    nc.tensor.matmul(
                        pse2,
                        w2e[:, k, :],
                        heT[le][:, k, :],
                        start=(k == 0), stop=(k == KO2 - 1))
                nc.scalar.activation(
                    obeT[:, dc, le * BATCH:(le + 1) * BATCH], pse2,
                    AF.Identity, bias=b2e_sb[le][:, dc:dc + 1])

            chs = {}
            for dc in range(3):
                chs[dc] = fill_w2ch(dc)

            def chunk(dc):
                shared_l2_chunk(dc, chs[dc])
                if dc + 3 < 8:
                    chs[dc + 3] = fill_w2ch(dc + 3)

            # 2 expert units between shared chunks; all 16 done before the
            # last chunk so the ns write (and the tail) hides under ch7
            units = [(le, dc) for le in range(E_PER_CORE) for dc in range(KO1)]
            for sc in range(7):
                chunk(sc)
                n = 2 if sc < 6 else 4
                for le, dc in units[2 * sc:2 * sc + n]:
                    expert_l2_unit(le, dc)
            nc.sync.dma_start(out=outnsT[:], in_=obeT)
            chunk(7)

    nc.compile()
    return nc


def _get_nc():
    if "nc" not in _state:
        _state["nc"] = _build()
    return _state["nc"]


def _bf(a):
    return np.ascontiguousarray(np.asarray(a, dtype=np.float32).astype(BF16))


def _f32(a):
    return np.ascontiguousarray(np.asarray(a, dtype=np.float32))


def kernel(x, W1_seq, b1_seq, W2_seq, b2_seq, W1_ns, b1_ns, W2_ns, b2_ns,
           seq_token_count):
    from concourse.bass_utils import run_bass_kernel_spmd

    assert int(seq_token_count) == SEQ_TOK
    xb16 = np.asarray(x, dtype=np.float32).astype(BF16)
    W1sb = np.asarray(W1_seq, dtype=np.float32).astype(BF16)
    W2sb = np.asarray(W2_seq, dtype=np.float32).astype(BF16)
    W1nb = np.asarray(W1_ns, dtype=np.float32).astype(BF16)
    W2nb = np.asarray(W2_ns, dtype=np.float32).astype(BF16)
    b1_seq, b2_seq = _f32(b1_seq), _f32(b2_seq)
    b1_ns, b2_ns = _f32(b1_ns), _f32(b2_ns)

    nc = _get_nc()

    # host-side (lossless) re-layouts: contraction dim on partitions, then
    # piece-major packing so each device DMA is one contiguous 1MiB read
    # w1sp[fb, p, fs, kc, fj] = W1_seq[kc*128+p, fb*512+fs*128+fj]
    w1sp_h = np.ascontiguousarray(
        W1sb.reshape(KO1, P, FBLK, 4, 128).transpose(2, 1, 3, 0, 4))
    # w2sp[dc, p, kc, di] = W2_seq[kc*128+p, dc*128+di]
    w2sp_h = np.ascontiguousarray(
        W2sb.reshape(KO2, P, 8, 128).transpose(2, 1, 0, 3))
    b1s_h = np.ascontiguousarray(b1_seq.reshape(KO2, P).T)          # [P, KO2]
    b2s_h = np.ascontiguousarray(b2_seq.reshape(KO1, P).T)          # [P, KO1]

    in_maps = []
    for i in range(N_CORES):
        # xbp[tb, p, kc, ti] = x[i, tb*512+ti, kc*128+p]
        xT = xb16[i, :SEQ_TOK, :].T                                 # [D, T]
        xbp_h = np.ascontiguousarray(
            xT.reshape(KO1, P, TBLK, 512).transpose(2, 1, 0, 3))
        # xnsT[p, kc, le*8+b] = x[b, 1024 + 2i + le, kc*128+p]
        xns = xb16[:, SEQ_TOK + 2 * i:SEQ_TOK + 2 * i + 2, :]       # [B, 2, D]
        xnsT_h = np.ascontiguousarray(
            xns.transpose(2, 1, 0).reshape(KO1, P, E_PER_CORE, BATCH)
            .transpose(1, 0, 2, 3).reshape(P, KO1, E_PER_CORE * BATCH))
        # w1ep[le, fb, p, fs, kc, fj] = W1_ns[2i+le, kc*128+p, fb*512+fs*128+fj]
        w1ep_h = np.ascontiguousarray(
            W1nb[2 * i:2 * i + 2].reshape(E_PER_CORE, KO1, P, FBLK, 4, 128)
            .transpose(0, 3, 2, 4, 1, 5))
        # w2ep[le, dc, p, kc, di] = W2_ns[2i+le, kc*128+p, dc*128+di]
        w2ep_h = np.ascontiguousarray(
            W2nb[2 * i:2 * i + 2].reshape(E_PER_CORE, KO2, P, KO1, 128)
            .transpose(0, 3, 2, 1, 4))
        b1e_h = b1_ns[2 * i:2 * i + 2].reshape(E_PER_CORE, KO2, P)
        b2e_h = b2_ns[2 * i:2 * i + 2].reshape(E_PER_CORE, KO1, P)
        consts_h = np.ascontiguousarray(np.concatenate([
            b1s_h, b2s_h, b1e_h[0].T, b1e_h[1].T, b2e_h[0].T, b2e_h[1].T,
        ], axis=1))
        in_maps.append({
            "xbp": xbp_h, "xnsT": xnsT_h,
            "w1sp": w1sp_h, "w2sp": w2sp_h, "consts": consts_h,
            "w1ep": w1ep_h, "w2ep": w2ep_h,
        })

    trace = bool(int(os.environ.get("KERNEL_TRACE", "0")))
    kw = {}
    if trace:
        kw["trace"] = True
        tc_env = os.environ.get("KERNEL_TRACE_CORES", "0")
        kw["trace_cores"] = [int(c) for c in tc_env.split(",")]
    res = run_bass_kernel_spmd(nc, in_maps, list(range(N_CORES)), **kw)
    _state["last_result"] = res

    out = np.empty((BATCH, SEQ_LEN, D_MODEL), np.float32)
    for i in range(N_CORES):
        out[i, :SEQ_TOK, :] = res.results[i]["outsT"].astype(np.float32).T
        # outnsT[p, dc, le*8+b] = out[b, 1024+2i+le, dc*128+p]
        ns = (res.results[i]["outnsT"].astype(np.float32)
              .transpose(2, 1, 0).reshape(E_PER_CORE, BATCH, D_MODEL))
        out[:, SEQ_TOK + 2 * i, :] = ns[0]
        out[:, SEQ_TOK + 2 * i + 1, :] = ns[1]
    return out
